# revision 16
# baseline (speedup 1.0000x reference)
"""MatAnyone memory-readout kernel for 8 Trainium2 NeuronCores.

Math (per batch b):
  sim[t,n]  = (-a_sq + two_ab - b_sq)[t,n] * ms[t] / sqrt(CK)
  aff       = softmax_t(sim)
  R[c,n]    = sum_t mv[c,t] * aff[t,n]
  out[c,n]  = R[c,n] * p[n] + lv[c,n] * (1 - p[n])

Sharding: 8 cores = 2 batches x 4 query-pixel shards (n = HW/4 = 576 each).
Single interleaved pass over 144 t-tiles; the two n-halves of 288 share each
t-tile's weights and one exp op. PSUM: sim pair 2 banks + 4 R accumulators +
2 Z accumulators = 8 banks exactly. Softmax runs with t on partitions:
  sim matmul:  lhsT = [mk^2 ; mk] (K=128=2*CK), rhs = [-qe/8 ; qe*qk/4]
  psum       -= b_sq/8 (DVE, broadcast tile)
  E           = Exp(psum * ms_t)      (ACT, per-partition scale)
  R, Z        = matmuls contracting t, accumulated across all 144 t-tiles
  out         = R * (p/Z) + lv * (1-p)
Softmax max-subtraction is skipped: sim <= 0 always (negative weighted L2
distance), and max_t sim ~ 0, so exp never overflows and Z >= exp(max) is
well-scaled.
"""

import sys

for _p in ("/opt/trn_rl_repo", "/root/.axon_site/_ro/trn_rl_repo"):
    if _p not in sys.path:
        sys.path.insert(0, _p)

from contextlib import ExitStack

import numpy as np

import concourse.bass as bass
from concourse import mybir
from concourse.bacc import Bacc
from concourse.tile import TileContext
from concourse.bass_utils import run_bass_kernel_spmd

F32 = mybir.dt.float32
F32R = mybir.dt.float32r
EXP = mybir.ActivationFunctionType.Exp

B, CK, CV, T, H, W = 2, 64, 256, 8, 48, 48
HW = H * W            # 2304
THW = T * HW          # 18432
NCORE = HW // 4       # 576 query pixels per core
NH = NCORE // 2       # 288 per n-half (psum-bank sized)
TT = THW // 128       # 144 t-tiles
MKCH = 4              # t-tiles per streamed M2 chunk
SKEW = 2              # software-pipeline skew (tiles) between exp and readout

_CACHE = {}


def _f32r(ap):
    return ap.bitcast(F32R)


def build_program():
    nc = Bacc(name="matanyone_knn")

    cz_h = nc.declare_dram_parameter("c_onesz", [128, 2], F32R, isOutput=False)
    cb_h = nc.declare_dram_parameter("c_onesb", [1, 128], F32R, isOutput=False)
    ce_h = nc.declare_dram_parameter("c_eighth", [CK, 128], F32R, isOutput=False)
    qk_h = nc.declare_dram_parameter("qk", [CK, NCORE], F32, isOutput=False)
    qe_h = nc.declare_dram_parameter("qe", [CK, NCORE], F32, isOutput=False)
    mk_h = nc.declare_dram_parameter("mk", [CK, THW], F32R, isOutput=False)
    ms_h = nc.declare_dram_parameter("msT", [128, TT], F32, isOutput=False)
    mv_h = nc.declare_dram_parameter("mvT", [THW, CV], F32R, isOutput=False)
    lv_h = nc.declare_dram_parameter("lv", [CV, NCORE], F32, isOutput=False)
    p_h = nc.declare_dram_parameter("p", [1, NCORE], F32, isOutput=False)
    out_h = nc.declare_dram_parameter("out", [CV, NCORE], F32, isOutput=True)

    with TileContext(nc) as tc, ExitStack() as ctx:
        persist = ctx.enter_context(tc.tile_pool(name="persist", bufs=1))
        mvpool = ctx.enter_context(tc.tile_pool(name="mv", bufs=1))
        m2pool = ctx.enter_context(tc.tile_pool(name="m2", bufs=2))
        epool = ctx.enter_context(tc.tile_pool(name="E", bufs=SKEW + 2))
        dpool = ctx.enter_context(tc.tile_pool(name="D", bufs=2))
        ps_pair = ctx.enter_context(tc.tile_pool(name="pspair", bufs=1, space="PSUM"))
        ps_acc = ctx.enter_context(tc.tile_pool(name="psacc", bufs=1, space="PSUM"))

        # ---- constants / setup -------------------------------------------
        ones_z = persist.tile([128, 2], F32R, tag="ones_z")      # Z matmul lhsT
        nc.sync.dma_start(out=ones_z[:], in_=cz_h[:])
        ones_b = persist.tile([1, 128], F32R, tag="ones_b")      # K=1 broadcast lhsT
        nc.sync.dma_start(out=ones_b[:], in_=cb_h[:])
        eighth = persist.tile([CK, 128], F32R, tag="eighth")     # b_sq/8 lhsT
        nc.sync.dma_start(out=eighth[:], in_=ce_h[:])

        ms_sb = persist.tile([128, TT], F32, tag="ms")
        nc.sync.dma_start(out=ms_sb[:], in_=ms_h[:])
        p_sb = persist.tile([1, NCORE], F32, tag="p")
        nc.sync.dma_start(out=p_sb[:], in_=p_h[:])

        q_sb = persist.tile([128, NCORE], F32R, tag="q")
        bsq_sb = persist.tile([128, NCORE], F32, tag="bsq")

        with tc.tile_pool(name="setup", bufs=1) as setup:
            qk_sb = setup.tile([CK, NCORE], F32, tag="qk")
            nc.sync.dma_start(out=qk_sb[:], in_=qk_h[:])
            qe_sb = setup.tile([CK, NCORE], F32, tag="qe")
            nc.sync.dma_start(out=qe_sb[:], in_=qe_h[:])
            t1 = setup.tile([CK, NCORE], F32, tag="t1")
            t2 = setup.tile([CK, NCORE], F32R, tag="t2")

            # copy-then-mul keeps each DVE op to a single cross-engine wait
            nc.vector.tensor_copy(t1[:], qk_sb[:])
            nc.vector.tensor_mul(t1[:], t1[:], qe_sb[:])               # qe*qk
            nc.vector.tensor_scalar_mul(q_sb[0:CK, :], qe_sb[:], -0.125)
            nc.vector.tensor_scalar_mul(q_sb[CK:128, :], t1[:], 0.25)
            nc.vector.tensor_mul(t2[:], t1[:], qk_sb[:])               # qe*qk^2

            pb = ps_pair.tile([128, 1024], F32, tag="pair")
            nc.tensor.matmul(pb[:, 0:NH], eighth[:], t2[:, 0:NH],
                             start=True, stop=True)
            nc.tensor.matmul(pb[:, 512:512 + NH], eighth[:],
                             t2[:, NH:2 * NH], start=True, stop=True)
            nc.vector.tensor_copy(bsq_sb[:, 0:NH], pb[:, 0:NH])
            nc.vector.tensor_copy(bsq_sb[:, NH:2 * NH], pb[:, 512:512 + NH])

        fin = ctx.enter_context(tc.tile_pool(name="fin", bufs=1))
        lv0 = fin.tile([128, NCORE], F32, tag="lv0")
        nc.sync.dma_start(out=lv0[:], in_=lv_h[0:128, :])
        lv1 = fin.tile([128, NCORE], F32, tag="lv1")
        nc.sync.dma_start(out=lv1[:], in_=lv_h[128:256, :])

        # ---- resident mvT ------------------------------------------------
        mv_sb = mvpool.tile([128, TT * CV], F32R, tag="mvres")
        for g in range(9):
            src = mv_h[g * 2048:(g + 1) * 2048, :].rearrange("(j p) c -> p j c", p=128)
            dst = mv_sb[:, g * 16 * CV:(g + 1) * 16 * CV].rearrange(
                "p (j c) -> p j c", c=CV)
            nc.sync.dma_start(out=dst, in_=src)

        # ---- main interleaved pass -------------------------------------
        r_acc = {}
        for k in (0, 1):
            for hh in (0, 1):
                r_acc[k, hh] = ps_acc.tile([128, NH], F32, tag=f"r{k}{hh}",
                                           name=f"r{k}{hh}")
        z_acc = [ps_acc.tile([2, NH], F32, tag=f"z{hh}", name=f"z{hh}")
                 for hh in (0, 1)]

        e_tiles = {}
        m2c = None
        for t in range(TT + SKEW):
            if t < TT:
                if t % MKCH == 0:
                    m2c = m2pool.tile([128, 128 * MKCH], F32R, tag="m2c")
                    nc.sync.dma_start(
                        out=m2c[CK:128, :],
                        in_=mk_h[:, t * 128:(t + MKCH) * 128])
                    nc.gpsimd.tensor_mul(m2c[0:CK, :], m2c[CK:128, :].bitcast(F32),
                                         m2c[CK:128, :].bitcast(F32))
                pair = ps_pair.tile([128, 1024], F32, tag="pair")
                lw = m2c[:, (t % MKCH) * 128:(t % MKCH + 1) * 128]
                nc.tensor.matmul(pair[:, 0:NH], lw, q_sb[:, 0:NH],
                                 start=True, stop=True)
                nc.tensor.matmul(pair[:, 512:512 + NH], lw, q_sb[:, NH:2 * NH],
                                 start=True, stop=True)
                # D = pair - b_sq/8  (both halves, strided view -> contiguous D)
                pview = pair[:].rearrange("p (k x) -> p k x", x=512)[:, :, 0:NH]
                dt_ = dpool.tile([128, NCORE], F32, tag="D")
                dview = dt_[:].rearrange("p (k x) -> p k x", x=NH)
                bview = bsq_sb[:].rearrange("p (k x) -> p k x", x=NH)
                nc.vector.tensor_sub(dview, pview, bview)
                e = epool.tile([128, NCORE], F32R, tag="E")
                nc.scalar.activation(e[:], dt_[:], EXP, scale=ms_sb[:, t:t + 1])
                e_tiles[t] = e
            if t >= SKEW:
                tc_ = t - SKEW
                e = e_tiles.pop(tc_)
                st, sp = (tc_ == 0), (tc_ == TT - 1)
                for k in (0, 1):
                    lwk = mv_sb[:, tc_ * CV + k * 128:tc_ * CV + (k + 1) * 128]
                    for hh in (0, 1):
                        nc.tensor.matmul(r_acc[k, hh][:], lwk,
                                         e[:, hh * NH:(hh + 1) * NH],
                                         start=st, stop=sp)
                for hh in (0, 1):
                    nc.tensor.matmul(z_acc[hh][:], ones_z[:],
                                     e[:, hh * NH:(hh + 1) * NH],
                                     start=st, stop=sp)

        # ---- finalize ----------------------------------------------------
        rz = fin.tile([1, NCORE], F32, tag="rz")
        nc.vector.reciprocal(rz[:, 0:NH], z_acc[0][0:1, :])
        nc.vector.reciprocal(rz[:, NH:2 * NH], z_acc[1][0:1, :])
        w1 = fin.tile([1, NCORE], F32R, tag="w1")
        nc.vector.tensor_mul(w1[:], rz[:], p_sb[:])            # p / Z
        w2 = fin.tile([1, NCORE], F32R, tag="w2")
        nc.vector.tensor_scalar_mul(w2[:], p_sb[:], -1.0)
        nc.vector.tensor_scalar_add(w2[:], w2[:], 1.0)         # 1 - p

        w1s = fin.tile([128, NCORE], F32, tag="w1s")
        w2s = fin.tile([128, NCORE], F32, tag="w2s")
        for w, ws in ((w1, w1s), (w2, w2s)):
            wps = ps_pair.tile([128, 1024], F32, tag="pair")
            nc.tensor.matmul(wps[:, 0:NH], ones_b[:], w[:, 0:NH],
                             start=True, stop=True)
            nc.tensor.matmul(wps[:, 512:512 + NH], ones_b[:], w[:, NH:2 * NH],
                             start=True, stop=True)
            wv = ws[:].rearrange("p (k x) -> p k x", x=NH)
            pv = wps[:].rearrange("p (k x) -> p k x", x=512)[:, :, 0:NH]
            nc.vector.tensor_copy(wv, pv)

        for k, lvt in ((0, lv0), (1, lv1)):
            o = fin.tile([128, NCORE], F32, tag="O", bufs=2)
            tmp = fin.tile([128, NCORE], F32, tag="tmp", bufs=2)
            for hh in (0, 1):
                s = slice(hh * NH, (hh + 1) * NH)
                nc.vector.tensor_mul(o[:, s], r_acc[k, hh][:], w1s[:, s])
            nc.vector.tensor_mul(tmp[:], lvt[:], w2s[:])
            nc.vector.tensor_add(o[:], o[:], tmp[:])
            nc.sync.dma_start(out=out_h[k * 128:(k + 1) * 128, :], in_=o[:])

    nc.finalize()
    return nc


def _get_program():
    if "nc" not in _CACHE:
        _CACHE["nc"] = build_program()
    return _CACHE["nc"]


def _make_in_maps(query_key, query_selection, memory_key, memory_shrinkage,
                  msk_value, uncert_prob):
    qk = np.asarray(query_key, np.float32).reshape(B, CK, HW)
    qe = np.asarray(query_selection, np.float32).reshape(B, CK, HW)
    mk = np.asarray(memory_key, np.float32).reshape(B, CK, THW)
    ms = np.asarray(memory_shrinkage, np.float32).reshape(B, THW)
    mv = np.asarray(msk_value, np.float32).reshape(B, CV, THW)
    lv = np.asarray(msk_value, np.float32).reshape(B, CV, T, HW)[:, :, T - 1, :]
    p = np.asarray(uncert_prob, np.float32).reshape(B, HW)

    in_maps = []
    for core in range(8):
        b, s = divmod(core, 4)
        sl = slice(s * NCORE, (s + 1) * NCORE)
        in_maps.append({
            "c_onesz": np.ones((128, 2), np.float32),
            "c_onesb": np.ones((1, 128), np.float32),
            "c_eighth": np.full((CK, 128), 0.125, np.float32),
            "qk": np.ascontiguousarray(qk[b, :, sl]),
            "qe": np.ascontiguousarray(qe[b, :, sl]),
            "mk": np.ascontiguousarray(mk[b]),
            "msT": np.ascontiguousarray(ms[b].reshape(TT, 128).T),
            "mvT": np.ascontiguousarray(mv[b].T),
            "lv": np.ascontiguousarray(lv[b, :, sl]),
            "p": np.ascontiguousarray(p[b, sl]).reshape(1, NCORE),
        })
    return in_maps


def kernel(**inputs):
    nc = _get_program()
    in_maps = _make_in_maps(**inputs)
    res = run_bass_kernel_spmd(nc, in_maps, list(range(8)))
    out = np.empty((B, 1, CV, HW), np.float32)
    for core in range(8):
        b, s = divmod(core, 4)
        out[b, 0, :, s * NCORE:(s + 1) * NCORE] = res.results[core]["out"]
    return out.reshape(B, 1, CV, H, W)


if __name__ == "__main__":
    rng = np.random.default_rng(0)
    dummy = {
        "query_key": rng.standard_normal((B, CK, H, W), np.float32),
        "query_selection": rng.random((B, CK, H, W), np.float32),
        "memory_key": rng.standard_normal((B, CK, T, H, W), np.float32),
        "memory_shrinkage": rng.random((B, 1, T, H, W), np.float32),
        "msk_value": rng.standard_normal((B, 1, CV, T, H, W), np.float32),
        "uncert_prob": rng.random((B, 1, H, W), np.float32),
    }
    out = kernel(**dummy)
    print("out", out.shape, out.dtype, float(np.abs(out).mean()))


# revision 17
# speedup vs baseline: 1.2371x; 1.2371x over previous
"""MatAnyone memory-readout kernel for 8 Trainium2 NeuronCores.

Math (per batch b):
  sim[t,n]  = (-a_sq + two_ab - b_sq)[t,n] * ms[t] / sqrt(CK)
  aff       = softmax_t(sim)
  R[c,n]    = sum_t mv[c,t] * aff[t,n]
  out[c,n]  = R[c,n] * p[n] + lv[c,n] * (1 - p[n])

Sharding: 8 cores = 2 batches x 4 query-pixel shards (n = HW/4 = 576 each).
Single interleaved pass over 144 t-tiles; the two n-halves of 288 share each
t-tile's weights and one exp op. PSUM: sim pair 2 banks + 4 R accumulators +
2 Z accumulators = 8 banks exactly. Softmax runs with t on partitions:
  sim matmul:  lhsT = [mk^2 ; mk] (K=128=2*CK), rhs = [-qe/8 ; qe*qk/4]
  psum       -= b_sq/8 (DVE, broadcast tile)
  E           = Exp(psum * ms_t)      (ACT, per-partition scale)
  R, Z        = matmuls contracting t, accumulated across all 144 t-tiles
  out         = R * (p/Z) + lv * (1-p)
Softmax max-subtraction is skipped: sim <= 0 always (negative weighted L2
distance), and max_t sim ~ 0, so exp never overflows and Z >= exp(max) is
well-scaled.
"""

import sys

for _p in ("/opt/trn_rl_repo", "/root/.axon_site/_ro/trn_rl_repo"):
    if _p not in sys.path:
        sys.path.insert(0, _p)

from contextlib import ExitStack

import numpy as np
import ml_dtypes

import concourse.bass as bass
from concourse import mybir
from concourse.bacc import Bacc
from concourse.tile import TileContext
from concourse.bass_utils import run_bass_kernel_spmd

F32 = mybir.dt.float32
F32R = mybir.dt.float32r
BF16 = mybir.dt.bfloat16
EXP = mybir.ActivationFunctionType.Exp

B, CK, CV, T, H, W = 2, 64, 256, 8, 48, 48
HW = H * W            # 2304
THW = T * HW          # 18432
NCORE = HW // 4       # 576 query pixels per core
NH = NCORE // 2       # 288 per n-half (psum-bank sized)
TT = THW // 128       # 144 t-tiles
MKCH = 4              # t-tiles per streamed M2 chunk
SKEW = 2              # software-pipeline skew (tiles) between exp and readout

_CACHE = {}


def _f32r(ap):
    return ap.bitcast(F32R)


def build_program():
    nc = Bacc(name="matanyone_knn")

    cz_h = nc.declare_dram_parameter("c_onesz", [128, 2], BF16, isOutput=False)
    cb_h = nc.declare_dram_parameter("c_onesb", [1, 128], F32R, isOutput=False)
    ce_h = nc.declare_dram_parameter("c_eighth", [CK, 128], F32R, isOutput=False)
    qk_h = nc.declare_dram_parameter("qk", [CK, NCORE], F32, isOutput=False)
    qe_h = nc.declare_dram_parameter("qe", [CK, NCORE], F32, isOutput=False)
    mk_h = nc.declare_dram_parameter("mk", [CK, THW], F32R, isOutput=False)
    ms_h = nc.declare_dram_parameter("msT", [128, TT], F32, isOutput=False)
    mv_h = nc.declare_dram_parameter("mvT", [THW, CV], BF16, isOutput=False)
    lv_h = nc.declare_dram_parameter("lv", [CV, NCORE], F32, isOutput=False)
    p_h = nc.declare_dram_parameter("p", [1, NCORE], F32, isOutput=False)
    out_h = nc.declare_dram_parameter("out", [CV, NCORE], F32, isOutput=True)

    with TileContext(nc) as tc, ExitStack() as ctx:
        persist = ctx.enter_context(tc.tile_pool(name="persist", bufs=1))
        mvpool = ctx.enter_context(tc.tile_pool(name="mv", bufs=1))
        m2pool = ctx.enter_context(tc.tile_pool(name="m2", bufs=2))
        epool = ctx.enter_context(tc.tile_pool(name="E", bufs=SKEW + 2))
        dpool = ctx.enter_context(tc.tile_pool(name="D", bufs=2))
        ps_sim = ctx.enter_context(tc.tile_pool(name="pssim", bufs=2, space="PSUM"))
        ps_acc = ctx.enter_context(tc.tile_pool(name="psacc", bufs=1, space="PSUM"))

        # ---- constants / setup -------------------------------------------
        ones_z = persist.tile([128, 2], BF16, tag="ones_z")      # Z matmul lhsT
        nc.sync.dma_start(out=ones_z[:], in_=cz_h[:])
        ones_b = persist.tile([1, 128], F32R, tag="ones_b")      # K=1 broadcast lhsT
        nc.sync.dma_start(out=ones_b[:], in_=cb_h[:])
        eighth = persist.tile([CK, 128], F32R, tag="eighth")     # b_sq/8 lhsT
        nc.sync.dma_start(out=eighth[:], in_=ce_h[:])

        ms_sb = persist.tile([128, TT], F32, tag="ms")
        nc.sync.dma_start(out=ms_sb[:], in_=ms_h[:])
        p_sb = persist.tile([1, NCORE], F32, tag="p")
        nc.sync.dma_start(out=p_sb[:], in_=p_h[:])

        q_sb = persist.tile([128, NCORE], F32R, tag="q")
        bsq_sb = persist.tile([128, NCORE], F32, tag="bsq")

        with tc.tile_pool(name="setup", bufs=1) as setup:
            qk_sb = setup.tile([CK, NCORE], F32, tag="qk")
            nc.sync.dma_start(out=qk_sb[:], in_=qk_h[:])
            qe_sb = setup.tile([CK, NCORE], F32, tag="qe")
            nc.sync.dma_start(out=qe_sb[:], in_=qe_h[:])
            t1 = setup.tile([CK, NCORE], F32, tag="t1")
            t2 = setup.tile([CK, NCORE], F32R, tag="t2")

            # copy-then-mul keeps each DVE op to a single cross-engine wait
            nc.vector.tensor_copy(t1[:], qk_sb[:])
            nc.vector.tensor_mul(t1[:], t1[:], qe_sb[:])               # qe*qk
            nc.vector.tensor_scalar_mul(q_sb[0:CK, :], qe_sb[:], -0.125)
            nc.vector.tensor_scalar_mul(q_sb[CK:128, :], t1[:], 0.25)
            nc.vector.tensor_mul(t2[:], t1[:], qk_sb[:])               # qe*qk^2

            for hh in (0, 1):
                pb = ps_sim.tile([128, NH], F32, tag="sim", name=f"pb{hh}")
                nc.tensor.matmul(pb[:], eighth[:], t2[:, hh * NH:(hh + 1) * NH],
                                 start=True, stop=True)
                nc.vector.tensor_copy(bsq_sb[:, hh * NH:(hh + 1) * NH], pb[:])

        fin = ctx.enter_context(tc.tile_pool(name="fin", bufs=1))
        lv0 = fin.tile([128, NCORE], F32, tag="lv0")
        nc.sync.dma_start(out=lv0[:], in_=lv_h[0:128, :])
        lv1 = fin.tile([128, NCORE], F32, tag="lv1")
        nc.sync.dma_start(out=lv1[:], in_=lv_h[128:256, :])

        # ---- resident mvT ------------------------------------------------
        mv_sb = mvpool.tile([128, TT * CV], BF16, tag="mvres")
        for g in range(9):
            src = mv_h[g * 2048:(g + 1) * 2048, :].rearrange("(j p) c -> p j c", p=128)
            dst = mv_sb[:, g * 16 * CV:(g + 1) * 16 * CV].rearrange(
                "p (j c) -> p j c", c=CV)
            nc.sync.dma_start(out=dst, in_=src)

        # ---- main interleaved pass -------------------------------------
        r_acc = {}
        for k in (0, 1):
            for hh in (0, 1):
                r_acc[k, hh] = ps_acc.tile([128, NH], F32, tag=f"r{k}{hh}",
                                           name=f"r{k}{hh}")
        z_acc = [ps_acc.tile([2, NH], F32, tag=f"z{hh}", name=f"z{hh}")
                 for hh in (0, 1)]

        e_tiles = {}
        m2c = None
        for t in range(TT + SKEW):
            if t < TT:
                if t % MKCH == 0:
                    m2c = m2pool.tile([128, 128 * MKCH], F32R, tag="m2c")
                    nc.sync.dma_start(
                        out=m2c[CK:128, :],
                        in_=mk_h[:, t * 128:(t + MKCH) * 128])
                    nc.gpsimd.tensor_mul(m2c[0:CK, :], m2c[CK:128, :].bitcast(F32),
                                         m2c[CK:128, :].bitcast(F32))
                lw = m2c[:, (t % MKCH) * 128:(t % MKCH + 1) * 128]
                dt_ = dpool.tile([128, NCORE], F32, tag="D")
                for hh in (0, 1):
                    s = slice(hh * NH, (hh + 1) * NH)
                    sim = ps_sim.tile([128, NH], F32, tag="sim", name=f"sim{hh}")
                    nc.tensor.matmul(sim[:], lw, q_sb[:, s],
                                     start=True, stop=True)
                    nc.vector.tensor_sub(dt_[:, s], sim[:], bsq_sb[:, s])
                e = epool.tile([128, NCORE], BF16, tag="E")
                nc.scalar.activation(e[:], dt_[:], EXP, scale=ms_sb[:, t:t + 1])
                e_tiles[t] = e
            if t >= SKEW:
                tc_ = t - SKEW
                e = e_tiles.pop(tc_)
                st, sp = (tc_ == 0), (tc_ == TT - 1)
                for k in (0, 1):
                    lwk = mv_sb[:, tc_ * CV + k * 128:tc_ * CV + (k + 1) * 128]
                    for hh in (0, 1):
                        nc.tensor.matmul(r_acc[k, hh][:], lwk,
                                         e[:, hh * NH:(hh + 1) * NH],
                                         start=st, stop=sp)
                for hh in (0, 1):
                    nc.tensor.matmul(z_acc[hh][:], ones_z[:],
                                     e[:, hh * NH:(hh + 1) * NH],
                                     start=st, stop=sp)

        # ---- finalize ----------------------------------------------------
        rz = fin.tile([1, NCORE], F32, tag="rz")
        nc.vector.reciprocal(rz[:, 0:NH], z_acc[0][0:1, :])
        nc.vector.reciprocal(rz[:, NH:2 * NH], z_acc[1][0:1, :])
        w1 = fin.tile([1, NCORE], F32R, tag="w1")
        nc.vector.tensor_mul(w1[:], rz[:], p_sb[:])            # p / Z
        w2 = fin.tile([1, NCORE], F32R, tag="w2")
        nc.vector.tensor_scalar_mul(w2[:], p_sb[:], -1.0)
        nc.vector.tensor_scalar_add(w2[:], w2[:], 1.0)         # 1 - p

        w1s = fin.tile([128, NCORE], F32, tag="w1s")
        w2s = fin.tile([128, NCORE], F32, tag="w2s")
        for w, ws in ((w1, w1s), (w2, w2s)):
            for hh in (0, 1):
                s = slice(hh * NH, (hh + 1) * NH)
                wps = ps_sim.tile([128, NH], F32, tag="sim", name=f"wps{hh}")
                nc.tensor.matmul(wps[:], ones_b[:], w[:, s],
                                 start=True, stop=True)
                nc.vector.tensor_copy(ws[:, s], wps[:])

        for k, lvt in ((0, lv0), (1, lv1)):
            o = fin.tile([128, NCORE], F32, tag="O", bufs=2)
            tmp = fin.tile([128, NCORE], F32, tag="tmp", bufs=2)
            for hh in (0, 1):
                s = slice(hh * NH, (hh + 1) * NH)
                nc.vector.tensor_mul(o[:, s], r_acc[k, hh][:], w1s[:, s])
            nc.vector.tensor_mul(tmp[:], lvt[:], w2s[:])
            nc.vector.tensor_add(o[:], o[:], tmp[:])
            nc.sync.dma_start(out=out_h[k * 128:(k + 1) * 128, :], in_=o[:])

    nc.finalize()
    return nc


def _get_program():
    if "nc" not in _CACHE:
        _CACHE["nc"] = build_program()
    return _CACHE["nc"]


def _make_in_maps(query_key, query_selection, memory_key, memory_shrinkage,
                  msk_value, uncert_prob):
    qk = np.asarray(query_key, np.float32).reshape(B, CK, HW)
    qe = np.asarray(query_selection, np.float32).reshape(B, CK, HW)
    mk = np.asarray(memory_key, np.float32).reshape(B, CK, THW)
    ms = np.asarray(memory_shrinkage, np.float32).reshape(B, THW)
    mv = np.asarray(msk_value, np.float32).reshape(B, CV, THW)
    lv = np.asarray(msk_value, np.float32).reshape(B, CV, T, HW)[:, :, T - 1, :]
    p = np.asarray(uncert_prob, np.float32).reshape(B, HW)

    in_maps = []
    for core in range(8):
        b, s = divmod(core, 4)
        sl = slice(s * NCORE, (s + 1) * NCORE)
        in_maps.append({
            "c_onesz": np.ones((128, 2), ml_dtypes.bfloat16),
            "c_onesb": np.ones((1, 128), np.float32),
            "c_eighth": np.full((CK, 128), 0.125, np.float32),
            "qk": np.ascontiguousarray(qk[b, :, sl]),
            "qe": np.ascontiguousarray(qe[b, :, sl]),
            "mk": np.ascontiguousarray(mk[b]),
            "msT": np.ascontiguousarray(ms[b].reshape(TT, 128).T),
            "mvT": np.ascontiguousarray(mv[b].T).astype(ml_dtypes.bfloat16),
            "lv": np.ascontiguousarray(lv[b, :, sl]),
            "p": np.ascontiguousarray(p[b, sl]).reshape(1, NCORE),
        })
    return in_maps


def kernel(**inputs):
    nc = _get_program()
    in_maps = _make_in_maps(**inputs)
    res = run_bass_kernel_spmd(nc, in_maps, list(range(8)))
    out = np.empty((B, 1, CV, HW), np.float32)
    for core in range(8):
        b, s = divmod(core, 4)
        out[b, 0, :, s * NCORE:(s + 1) * NCORE] = res.results[core]["out"]
    return out.reshape(B, 1, CV, H, W)


if __name__ == "__main__":
    rng = np.random.default_rng(0)
    dummy = {
        "query_key": rng.standard_normal((B, CK, H, W), np.float32),
        "query_selection": rng.random((B, CK, H, W), np.float32),
        "memory_key": rng.standard_normal((B, CK, T, H, W), np.float32),
        "memory_shrinkage": rng.random((B, 1, T, H, W), np.float32),
        "msk_value": rng.standard_normal((B, 1, CV, T, H, W), np.float32),
        "uncert_prob": rng.random((B, 1, H, W), np.float32),
    }
    out = kernel(**dummy)
    print("out", out.shape, out.dtype, float(np.abs(out).mean()))


# revision 18
# speedup vs baseline: 1.3886x; 1.1224x over previous
"""MatAnyone memory-readout kernel for 8 Trainium2 NeuronCores.

Math (per batch b):
  sim[t,n]  = (-a_sq + two_ab - b_sq)[t,n] * ms[t] / sqrt(CK)
  aff       = softmax_t(sim)
  R[c,n]    = sum_t mv[c,t] * aff[t,n]
  out[c,n]  = R[c,n] * p[n] + lv[c,n] * (1 - p[n])

Sharding: 8 cores = 2 batches x 4 query-pixel shards (n = HW/4 = 576 each).
Single interleaved pass over 144 t-tiles; the two n-halves of 288 share each
t-tile's weights and one exp op. PSUM: sim pair 2 banks + 4 R accumulators +
2 Z accumulators = 8 banks exactly. Softmax runs with t on partitions:
  sim matmul:  lhsT = [mk^2 ; mk] (K=128=2*CK), rhs = [-qe/8 ; qe*qk/4]
  psum       -= b_sq/8 (DVE, broadcast tile)
  E           = Exp(psum * ms_t)      (ACT, per-partition scale)
  R, Z        = matmuls contracting t, accumulated across all 144 t-tiles
  out         = R * (p/Z) + lv * (1-p)
Softmax max-subtraction is skipped: sim <= 0 always (negative weighted L2
distance), and max_t sim ~ 0, so exp never overflows and Z >= exp(max) is
well-scaled.
"""

import sys

for _p in ("/opt/trn_rl_repo", "/root/.axon_site/_ro/trn_rl_repo"):
    if _p not in sys.path:
        sys.path.insert(0, _p)

from contextlib import ExitStack

import numpy as np
import ml_dtypes

import concourse.bass as bass
from concourse import mybir
from concourse.bacc import Bacc
from concourse.tile import TileContext
from concourse.bass_utils import run_bass_kernel_spmd

F32 = mybir.dt.float32
F32R = mybir.dt.float32r
BF16 = mybir.dt.bfloat16
EXP = mybir.ActivationFunctionType.Exp

B, CK, CV, T, H, W = 2, 64, 256, 8, 48, 48
HW = H * W            # 2304
THW = T * HW          # 18432
NCORE = HW // 4       # 576 query pixels per core
NH = NCORE // 2       # 288 per n-half (psum-bank sized)
TT = THW // 128       # 144 t-tiles
MKCH = 4              # t-tiles per streamed M2 chunk
SKEW = 3              # software-pipeline skew (tiles) between exp and readout

_CACHE = {}


def _f32r(ap):
    return ap.bitcast(F32R)


def build_program():
    nc = Bacc(name="matanyone_knn")

    cz_h = nc.declare_dram_parameter("c_onesz", [128, 2], BF16, isOutput=False)
    cb_h = nc.declare_dram_parameter("c_onesb", [1, 128], F32R, isOutput=False)
    ce_h = nc.declare_dram_parameter("c_eighth", [CK, 128], F32R, isOutput=False)
    qk_h = nc.declare_dram_parameter("qk", [CK, NCORE], F32, isOutput=False)
    qe_h = nc.declare_dram_parameter("qe", [CK, NCORE], F32, isOutput=False)
    mk_h = nc.declare_dram_parameter("mk", [CK, THW], F32R, isOutput=False)
    ms_h = nc.declare_dram_parameter("msT", [128, TT], F32, isOutput=False)
    mv_h = nc.declare_dram_parameter("mvT", [THW, CV], BF16, isOutput=False)
    lv_h = nc.declare_dram_parameter("lv", [CV, NCORE], F32, isOutput=False)
    p_h = nc.declare_dram_parameter("p", [1, NCORE], F32, isOutput=False)
    out_h = nc.declare_dram_parameter("out", [CV, NCORE], F32, isOutput=True)

    with TileContext(nc) as tc, ExitStack() as ctx:
        persist = ctx.enter_context(tc.tile_pool(name="persist", bufs=1))
        mvpool = ctx.enter_context(tc.tile_pool(name="mv", bufs=1))
        m2pool = ctx.enter_context(tc.tile_pool(name="m2", bufs=2))
        epool = ctx.enter_context(tc.tile_pool(name="E", bufs=SKEW + 2))
        dpool = ctx.enter_context(tc.tile_pool(name="D", bufs=2))
        ps_sim = ctx.enter_context(tc.tile_pool(name="pssim", bufs=2, space="PSUM"))
        ps_acc = ctx.enter_context(tc.tile_pool(name="psacc", bufs=1, space="PSUM"))

        # ---- constants / setup -------------------------------------------
        ones_z = persist.tile([128, 2], BF16, tag="ones_z")      # Z matmul lhsT
        nc.sync.dma_start(out=ones_z[:], in_=cz_h[:])
        ones_b = persist.tile([1, 128], F32R, tag="ones_b")      # K=1 broadcast lhsT
        nc.sync.dma_start(out=ones_b[:], in_=cb_h[:])
        eighth = persist.tile([CK, 128], F32R, tag="eighth")     # b_sq/8 lhsT
        nc.sync.dma_start(out=eighth[:], in_=ce_h[:])

        ms_sb = persist.tile([128, TT], F32, tag="ms")
        nc.sync.dma_start(out=ms_sb[:], in_=ms_h[:])
        p_sb = persist.tile([1, NCORE], F32, tag="p")
        nc.sync.dma_start(out=p_sb[:], in_=p_h[:])

        q_sb = persist.tile([128, NCORE], F32R, tag="q")
        bsq_sb = persist.tile([128, NCORE], F32, tag="bsq")

        with tc.tile_pool(name="setup", bufs=1) as setup:
            qk_sb = setup.tile([CK, NCORE], F32, tag="qk")
            nc.sync.dma_start(out=qk_sb[:], in_=qk_h[:])
            qe_sb = setup.tile([CK, NCORE], F32, tag="qe")
            nc.sync.dma_start(out=qe_sb[:], in_=qe_h[:])
            t1 = setup.tile([CK, NCORE], F32, tag="t1")
            t2 = setup.tile([CK, NCORE], F32R, tag="t2")

            # copy-then-mul keeps each DVE op to a single cross-engine wait
            nc.vector.tensor_copy(t1[:], qk_sb[:])
            nc.vector.tensor_mul(t1[:], t1[:], qe_sb[:])               # qe*qk
            nc.vector.tensor_scalar_mul(q_sb[0:CK, :], qe_sb[:], -0.125)
            nc.vector.tensor_scalar_mul(q_sb[CK:128, :], t1[:], 0.25)
            nc.vector.tensor_mul(t2[:], t1[:], qk_sb[:])               # qe*qk^2

            for hh in (0, 1):
                pb = ps_sim.tile([128, NH], F32, tag="sim", name=f"pb{hh}")
                nc.tensor.matmul(pb[:], eighth[:], t2[:, hh * NH:(hh + 1) * NH],
                                 start=True, stop=True)
                nc.vector.tensor_copy(bsq_sb[:, hh * NH:(hh + 1) * NH], pb[:])

        fin = ctx.enter_context(tc.tile_pool(name="fin", bufs=1))
        lv0 = fin.tile([128, NCORE], F32, tag="lv0")
        nc.sync.dma_start(out=lv0[:], in_=lv_h[0:128, :])
        lv1 = fin.tile([128, NCORE], F32, tag="lv1")
        nc.sync.dma_start(out=lv1[:], in_=lv_h[128:256, :])

        # ---- resident mvT (chunks DMA'd inside the main loop) -----------
        mv_sb = mvpool.tile([128, TT * CV], BF16, tag="mvres")

        def load_mv_chunk(g):
            src = mv_h[g * 2048:(g + 1) * 2048, :].rearrange(
                "(j p) c -> p j c", p=128)
            dst = mv_sb[:, g * 16 * CV:(g + 1) * 16 * CV].rearrange(
                "p (j c) -> p j c", c=CV)
            nc.sync.dma_start(out=dst, in_=src)

        # ---- main interleaved pass -------------------------------------
        r_acc = {}
        for k in (0, 1):
            for hh in (0, 1):
                r_acc[k, hh] = ps_acc.tile([128, NH], F32, tag=f"r{k}{hh}",
                                           name=f"r{k}{hh}")
        z_acc = [ps_acc.tile([2, NH], F32, tag=f"z{hh}", name=f"z{hh}")
                 for hh in (0, 1)]

        e_tiles = {}
        m2c = None
        for t in range(TT + SKEW):
            if t < TT:
                if t % 16 == 0:
                    load_mv_chunk(t // 16)
                if t % MKCH == 0:
                    m2c = m2pool.tile([128, 128 * MKCH], F32R, tag="m2c")
                    nc.sync.dma_start(
                        out=m2c[CK:128, :],
                        in_=mk_h[:, t * 128:(t + MKCH) * 128])
                    nc.gpsimd.tensor_mul(m2c[0:CK, :], m2c[CK:128, :].bitcast(F32),
                                         m2c[CK:128, :].bitcast(F32))
                lw = m2c[:, (t % MKCH) * 128:(t % MKCH + 1) * 128]
                dt_ = dpool.tile([128, NCORE], F32, tag="D")
                for hh in (0, 1):
                    s = slice(hh * NH, (hh + 1) * NH)
                    sim = ps_sim.tile([128, NH], F32, tag="sim", name=f"sim{hh}")
                    nc.tensor.matmul(sim[:], lw, q_sb[:, s],
                                     start=True, stop=True)
                    nc.vector.tensor_sub(dt_[:, s], sim[:], bsq_sb[:, s])
                e = epool.tile([128, NCORE], BF16, tag="E")
                nc.scalar.activation(e[:], dt_[:], EXP, scale=ms_sb[:, t:t + 1])
                e_tiles[t] = e
            if t >= SKEW:
                tc_ = t - SKEW
                e = e_tiles.pop(tc_)
                st, sp = (tc_ == 0), (tc_ == TT - 1)
                for k in (0, 1):
                    lwk = mv_sb[:, tc_ * CV + k * 128:tc_ * CV + (k + 1) * 128]
                    for hh in (0, 1):
                        nc.tensor.matmul(r_acc[k, hh][:], lwk,
                                         e[:, hh * NH:(hh + 1) * NH],
                                         start=st, stop=sp)
                for hh in (0, 1):
                    nc.tensor.matmul(z_acc[hh][:], ones_z[:],
                                     e[:, hh * NH:(hh + 1) * NH],
                                     start=st, stop=sp)

        # ---- finalize ----------------------------------------------------
        rz = fin.tile([1, NCORE], F32, tag="rz")
        nc.vector.reciprocal(rz[:, 0:NH], z_acc[0][0:1, :])
        nc.vector.reciprocal(rz[:, NH:2 * NH], z_acc[1][0:1, :])
        w1 = fin.tile([1, NCORE], F32R, tag="w1")
        nc.vector.tensor_mul(w1[:], rz[:], p_sb[:])            # p / Z
        w2 = fin.tile([1, NCORE], F32R, tag="w2")
        nc.vector.tensor_scalar_mul(w2[:], p_sb[:], -1.0)
        nc.vector.tensor_scalar_add(w2[:], w2[:], 1.0)         # 1 - p

        w1s = fin.tile([128, NCORE], F32, tag="w1s")
        w2s = fin.tile([128, NCORE], F32, tag="w2s")
        for w, ws in ((w1, w1s), (w2, w2s)):
            for hh in (0, 1):
                s = slice(hh * NH, (hh + 1) * NH)
                wps = ps_sim.tile([128, NH], F32, tag="sim", name=f"wps{hh}")
                nc.tensor.matmul(wps[:], ones_b[:], w[:, s],
                                 start=True, stop=True)
                nc.vector.tensor_copy(ws[:, s], wps[:])

        for k, lvt in ((0, lv0), (1, lv1)):
            o = fin.tile([128, NCORE], F32, tag="O", bufs=2)
            tmp = fin.tile([128, NCORE], F32, tag="tmp", bufs=2)
            for hh in (0, 1):
                s = slice(hh * NH, (hh + 1) * NH)
                nc.vector.tensor_mul(o[:, s], r_acc[k, hh][:], w1s[:, s])
            nc.vector.tensor_mul(tmp[:], lvt[:], w2s[:])
            nc.vector.tensor_add(o[:], o[:], tmp[:])
            nc.sync.dma_start(out=out_h[k * 128:(k + 1) * 128, :], in_=o[:])

    nc.finalize()
    return nc


def _get_program():
    if "nc" not in _CACHE:
        _CACHE["nc"] = build_program()
    return _CACHE["nc"]


def _make_in_maps(query_key, query_selection, memory_key, memory_shrinkage,
                  msk_value, uncert_prob):
    qk = np.asarray(query_key, np.float32).reshape(B, CK, HW)
    qe = np.asarray(query_selection, np.float32).reshape(B, CK, HW)
    mk = np.asarray(memory_key, np.float32).reshape(B, CK, THW)
    ms = np.asarray(memory_shrinkage, np.float32).reshape(B, THW)
    mv = np.asarray(msk_value, np.float32).reshape(B, CV, THW)
    lv = np.asarray(msk_value, np.float32).reshape(B, CV, T, HW)[:, :, T - 1, :]
    p = np.asarray(uncert_prob, np.float32).reshape(B, HW)

    in_maps = []
    for core in range(8):
        b, s = divmod(core, 4)
        sl = slice(s * NCORE, (s + 1) * NCORE)
        in_maps.append({
            "c_onesz": np.ones((128, 2), ml_dtypes.bfloat16),
            "c_onesb": np.ones((1, 128), np.float32),
            "c_eighth": np.full((CK, 128), 0.125, np.float32),
            "qk": np.ascontiguousarray(qk[b, :, sl]),
            "qe": np.ascontiguousarray(qe[b, :, sl]),
            "mk": np.ascontiguousarray(mk[b]),
            "msT": np.ascontiguousarray(ms[b].reshape(TT, 128).T),
            "mvT": np.ascontiguousarray(mv[b].T).astype(ml_dtypes.bfloat16),
            "lv": np.ascontiguousarray(lv[b, :, sl]),
            "p": np.ascontiguousarray(p[b, sl]).reshape(1, NCORE),
        })
    return in_maps


def kernel(**inputs):
    nc = _get_program()
    in_maps = _make_in_maps(**inputs)
    res = run_bass_kernel_spmd(nc, in_maps, list(range(8)))
    out = np.empty((B, 1, CV, HW), np.float32)
    for core in range(8):
        b, s = divmod(core, 4)
        out[b, 0, :, s * NCORE:(s + 1) * NCORE] = res.results[core]["out"]
    return out.reshape(B, 1, CV, H, W)


if __name__ == "__main__":
    rng = np.random.default_rng(0)
    dummy = {
        "query_key": rng.standard_normal((B, CK, H, W), np.float32),
        "query_selection": rng.random((B, CK, H, W), np.float32),
        "memory_key": rng.standard_normal((B, CK, T, H, W), np.float32),
        "memory_shrinkage": rng.random((B, 1, T, H, W), np.float32),
        "msk_value": rng.standard_normal((B, 1, CV, T, H, W), np.float32),
        "uncert_prob": rng.random((B, 1, H, W), np.float32),
    }
    out = kernel(**dummy)
    print("out", out.shape, out.dtype, float(np.abs(out).mean()))


# revision 19
# speedup vs baseline: 1.5193x; 1.0942x over previous
"""MatAnyone memory-readout kernel for 8 Trainium2 NeuronCores.

Math (per batch b):
  sim[t,n]  = (-a_sq + two_ab - b_sq)[t,n] * ms[t] / sqrt(CK)
  aff       = softmax_t(sim)
  R[c,n]    = sum_t mv[c,t] * aff[t,n]
  out[c,n]  = R[c,n] * p[n] + lv[c,n] * (1 - p[n])

Sharding: 8 cores = 2 batches x 4 query-pixel shards (n = HW/4 = 576 each).
Single interleaved pass over 144 t-tiles; the two n-halves of 288 share each
t-tile's weights and one exp op. PSUM: sim pair 2 banks + 4 R accumulators +
2 Z accumulators = 8 banks exactly. Softmax runs with t on partitions:
  sim matmul:  lhsT = [mk^2 ; mk] (K=128=2*CK), rhs = [-qe/8 ; qe*qk/4]
  psum       -= b_sq/8 (DVE, broadcast tile)
  E           = Exp(psum * ms_t)      (ACT, per-partition scale)
  R, Z        = matmuls contracting t, accumulated across all 144 t-tiles
  out         = R * (p/Z) + lv * (1-p)
Softmax max-subtraction is skipped: sim <= 0 always (negative weighted L2
distance), and max_t sim ~ 0, so exp never overflows and Z >= exp(max) is
well-scaled.
"""

import sys

for _p in ("/opt/trn_rl_repo", "/root/.axon_site/_ro/trn_rl_repo"):
    if _p not in sys.path:
        sys.path.insert(0, _p)

from contextlib import ExitStack

import numpy as np
import ml_dtypes

import concourse.bass as bass
from concourse import mybir
from concourse.bacc import Bacc
from concourse.tile import TileContext
from concourse.bass_utils import run_bass_kernel_spmd

F32 = mybir.dt.float32
F32R = mybir.dt.float32r
BF16 = mybir.dt.bfloat16
FP16 = mybir.dt.float16
EXP = mybir.ActivationFunctionType.Exp

B, CK, CV, T, H, W = 2, 64, 256, 8, 48, 48
HW = H * W            # 2304
THW = T * HW          # 18432
NCORE = HW // 4       # 576 query pixels per core
NH = NCORE // 2       # 288 per n-half (psum-bank sized)
TT = THW // 128       # 144 t-tiles
MKCH = 4              # t-tiles per streamed M2 chunk
SKEW = 3              # software-pipeline skew (tiles) between exp and readout

_CACHE = {}


def _f32r(ap):
    return ap.bitcast(F32R)


def build_program():
    nc = Bacc(name="matanyone_knn")

    cz_h = nc.declare_dram_parameter("c_onesz", [128, 2], BF16, isOutput=False)
    cb_h = nc.declare_dram_parameter("c_onesb", [1, 128], F32R, isOutput=False)
    ce_h = nc.declare_dram_parameter("c_eighth", [CK, 128], F32R, isOutput=False)
    qk_h = nc.declare_dram_parameter("qk", [CK, NCORE], F32, isOutput=False)
    qe_h = nc.declare_dram_parameter("qe", [CK, NCORE], F32, isOutput=False)
    mk_h = nc.declare_dram_parameter("mk", [CK, THW], FP16, isOutput=False)
    ms_h = nc.declare_dram_parameter("msT", [128, TT], F32, isOutput=False)
    mv_h = nc.declare_dram_parameter("mvT", [THW, CV], BF16, isOutput=False)
    lv_h = nc.declare_dram_parameter("lv", [CV, NCORE], F32, isOutput=False)
    p_h = nc.declare_dram_parameter("p", [1, NCORE], F32, isOutput=False)
    out_h = nc.declare_dram_parameter("out", [CV, NCORE], F32, isOutput=True)

    with TileContext(nc) as tc, ExitStack() as ctx:
        persist = ctx.enter_context(tc.tile_pool(name="persist", bufs=1))
        mvpool = ctx.enter_context(tc.tile_pool(name="mv", bufs=1))
        m2pool = ctx.enter_context(tc.tile_pool(name="m2", bufs=2))
        epool = ctx.enter_context(tc.tile_pool(name="E", bufs=SKEW + 2))
        dpool = ctx.enter_context(tc.tile_pool(name="D", bufs=2))
        ps_sim = ctx.enter_context(tc.tile_pool(name="pssim", bufs=2, space="PSUM"))
        ps_acc = ctx.enter_context(tc.tile_pool(name="psacc", bufs=1, space="PSUM"))

        # ---- constants / setup -------------------------------------------
        ones_z = persist.tile([128, 2], BF16, tag="ones_z")      # Z matmul lhsT
        nc.sync.dma_start(out=ones_z[:], in_=cz_h[:])
        ones_b = persist.tile([1, 128], F32R, tag="ones_b")      # K=1 broadcast lhsT
        nc.sync.dma_start(out=ones_b[:], in_=cb_h[:])
        eighth = persist.tile([CK, 128], F32R, tag="eighth")     # b_sq/8 lhsT
        nc.sync.dma_start(out=eighth[:], in_=ce_h[:])

        ms_sb = persist.tile([128, TT], F32, tag="ms")
        nc.sync.dma_start(out=ms_sb[:], in_=ms_h[:])
        p_sb = persist.tile([1, NCORE], F32, tag="p")
        nc.sync.dma_start(out=p_sb[:], in_=p_h[:])

        q_sb = persist.tile([128, NCORE], FP16, tag="q")
        bsq_sb = persist.tile([128, NCORE], F32, tag="bsq")

        with tc.tile_pool(name="setup", bufs=1) as setup:
            qk_sb = setup.tile([CK, NCORE], F32, tag="qk")
            nc.sync.dma_start(out=qk_sb[:], in_=qk_h[:])
            qe_sb = setup.tile([CK, NCORE], F32, tag="qe")
            nc.sync.dma_start(out=qe_sb[:], in_=qe_h[:])
            t1 = setup.tile([CK, NCORE], F32, tag="t1")
            t2 = setup.tile([CK, NCORE], F32R, tag="t2")

            # copy-then-mul keeps each DVE op to a single cross-engine wait
            nc.vector.tensor_copy(t1[:], qk_sb[:])
            nc.vector.tensor_mul(t1[:], t1[:], qe_sb[:])               # qe*qk
            nc.vector.tensor_scalar_mul(q_sb[0:CK, :], qe_sb[:], -0.125)
            nc.vector.tensor_scalar_mul(q_sb[CK:128, :], t1[:], 0.25)
            nc.vector.tensor_mul(t2[:], t1[:], qk_sb[:])               # qe*qk^2

            for hh in (0, 1):
                pb = ps_sim.tile([128, NH], F32, tag="sim", name=f"pb{hh}")
                nc.tensor.matmul(pb[:], eighth[:], t2[:, hh * NH:(hh + 1) * NH],
                                 start=True, stop=True)
                nc.vector.tensor_copy(bsq_sb[:, hh * NH:(hh + 1) * NH], pb[:])

        fin = ctx.enter_context(tc.tile_pool(name="fin", bufs=1))
        lv0 = fin.tile([128, NCORE], F32, tag="lv0")
        nc.sync.dma_start(out=lv0[:], in_=lv_h[0:128, :])
        lv1 = fin.tile([128, NCORE], F32, tag="lv1")
        nc.sync.dma_start(out=lv1[:], in_=lv_h[128:256, :])

        # ---- resident mvT (chunks DMA'd inside the main loop) -----------
        mv_sb = mvpool.tile([128, TT * CV], BF16, tag="mvres")

        def load_mv_chunk(g):
            src = mv_h[g * 2048:(g + 1) * 2048, :].rearrange(
                "(j p) c -> p j c", p=128)
            dst = mv_sb[:, g * 16 * CV:(g + 1) * 16 * CV].rearrange(
                "p (j c) -> p j c", c=CV)
            nc.sync.dma_start(out=dst, in_=src)

        # ---- main interleaved pass -------------------------------------
        r_acc = {}
        for k in (0, 1):
            for hh in (0, 1):
                r_acc[k, hh] = ps_acc.tile([128, NH], F32, tag=f"r{k}{hh}",
                                           name=f"r{k}{hh}")
        z_acc = [ps_acc.tile([2, NH], F32, tag=f"z{hh}", name=f"z{hh}")
                 for hh in (0, 1)]

        e_tiles = {}
        m2c = None
        for t in range(TT + SKEW):
            if t < TT:
                if t % 16 == 0:
                    load_mv_chunk(t // 16)
                if t % MKCH == 0:
                    m2c = m2pool.tile([128, 128 * MKCH], FP16, tag="m2c")
                    nc.sync.dma_start(
                        out=m2c[CK:128, :],
                        in_=mk_h[:, t * 128:(t + MKCH) * 128])
                    nc.gpsimd.tensor_mul(m2c[0:CK, :], m2c[CK:128, :],
                                         m2c[CK:128, :])
                lw = m2c[:, (t % MKCH) * 128:(t % MKCH + 1) * 128]
                dt_ = dpool.tile([128, NCORE], F32, tag="D")
                for hh in (0, 1):
                    s = slice(hh * NH, (hh + 1) * NH)
                    sim = ps_sim.tile([128, NH], F32, tag="sim", name=f"sim{hh}")
                    nc.tensor.matmul(sim[:], lw, q_sb[:, s],
                                     start=True, stop=True)
                    nc.vector.tensor_sub(dt_[:, s], sim[:], bsq_sb[:, s])
                e = epool.tile([128, NCORE], BF16, tag="E")
                nc.scalar.activation(e[:], dt_[:], EXP, scale=ms_sb[:, t:t + 1])
                e_tiles[t] = e
            if t >= SKEW:
                tc_ = t - SKEW
                e = e_tiles.pop(tc_)
                st, sp = (tc_ == 0), (tc_ == TT - 1)
                for k in (0, 1):
                    lwk = mv_sb[:, tc_ * CV + k * 128:tc_ * CV + (k + 1) * 128]
                    for hh in (0, 1):
                        nc.tensor.matmul(r_acc[k, hh][:], lwk,
                                         e[:, hh * NH:(hh + 1) * NH],
                                         start=st, stop=sp)
                for hh in (0, 1):
                    nc.tensor.matmul(z_acc[hh][:], ones_z[:],
                                     e[:, hh * NH:(hh + 1) * NH],
                                     start=st, stop=sp)

        # ---- finalize ----------------------------------------------------
        rz = fin.tile([1, NCORE], F32, tag="rz")
        nc.vector.reciprocal(rz[:, 0:NH], z_acc[0][0:1, :])
        nc.vector.reciprocal(rz[:, NH:2 * NH], z_acc[1][0:1, :])
        w1 = fin.tile([1, NCORE], F32R, tag="w1")
        nc.vector.tensor_mul(w1[:], rz[:], p_sb[:])            # p / Z
        w2 = fin.tile([1, NCORE], F32R, tag="w2")
        nc.vector.tensor_scalar_mul(w2[:], p_sb[:], -1.0)
        nc.vector.tensor_scalar_add(w2[:], w2[:], 1.0)         # 1 - p

        w1s = fin.tile([128, NCORE], F32, tag="w1s")
        w2s = fin.tile([128, NCORE], F32, tag="w2s")
        for w, ws in ((w1, w1s), (w2, w2s)):
            for hh in (0, 1):
                s = slice(hh * NH, (hh + 1) * NH)
                wps = ps_sim.tile([128, NH], F32, tag="sim", name=f"wps{hh}")
                nc.tensor.matmul(wps[:], ones_b[:], w[:, s],
                                 start=True, stop=True)
                nc.vector.tensor_copy(ws[:, s], wps[:])

        for k, lvt in ((0, lv0), (1, lv1)):
            o = fin.tile([128, NCORE], F32, tag="O", bufs=2)
            tmp = fin.tile([128, NCORE], F32, tag="tmp", bufs=2)
            for hh in (0, 1):
                s = slice(hh * NH, (hh + 1) * NH)
                nc.vector.tensor_mul(o[:, s], r_acc[k, hh][:], w1s[:, s])
            nc.vector.tensor_mul(tmp[:], lvt[:], w2s[:])
            nc.vector.tensor_add(o[:], o[:], tmp[:])
            nc.sync.dma_start(out=out_h[k * 128:(k + 1) * 128, :], in_=o[:])

    nc.finalize()
    return nc


def _get_program():
    if "nc" not in _CACHE:
        _CACHE["nc"] = build_program()
    return _CACHE["nc"]


def _make_in_maps(query_key, query_selection, memory_key, memory_shrinkage,
                  msk_value, uncert_prob):
    qk = np.asarray(query_key, np.float32).reshape(B, CK, HW)
    qe = np.asarray(query_selection, np.float32).reshape(B, CK, HW)
    mk = np.asarray(memory_key, np.float32).reshape(B, CK, THW)
    ms = np.asarray(memory_shrinkage, np.float32).reshape(B, THW)
    mv = np.asarray(msk_value, np.float32).reshape(B, CV, THW)
    lv = np.asarray(msk_value, np.float32).reshape(B, CV, T, HW)[:, :, T - 1, :]
    p = np.asarray(uncert_prob, np.float32).reshape(B, HW)

    in_maps = []
    for core in range(8):
        b, s = divmod(core, 4)
        sl = slice(s * NCORE, (s + 1) * NCORE)
        in_maps.append({
            "c_onesz": np.ones((128, 2), ml_dtypes.bfloat16),
            "c_onesb": np.ones((1, 128), np.float32),
            "c_eighth": np.full((CK, 128), 0.125, np.float32),
            "qk": np.ascontiguousarray(qk[b, :, sl]),
            "qe": np.ascontiguousarray(qe[b, :, sl]),
            "mk": np.ascontiguousarray(mk[b]).astype(np.float16),
            "msT": np.ascontiguousarray(ms[b].reshape(TT, 128).T),
            "mvT": np.ascontiguousarray(mv[b].T).astype(ml_dtypes.bfloat16),
            "lv": np.ascontiguousarray(lv[b, :, sl]),
            "p": np.ascontiguousarray(p[b, sl]).reshape(1, NCORE),
        })
    return in_maps


def kernel(**inputs):
    nc = _get_program()
    in_maps = _make_in_maps(**inputs)
    res = run_bass_kernel_spmd(nc, in_maps, list(range(8)))
    out = np.empty((B, 1, CV, HW), np.float32)
    for core in range(8):
        b, s = divmod(core, 4)
        out[b, 0, :, s * NCORE:(s + 1) * NCORE] = res.results[core]["out"]
    return out.reshape(B, 1, CV, H, W)


if __name__ == "__main__":
    rng = np.random.default_rng(0)
    dummy = {
        "query_key": rng.standard_normal((B, CK, H, W), np.float32),
        "query_selection": rng.random((B, CK, H, W), np.float32),
        "memory_key": rng.standard_normal((B, CK, T, H, W), np.float32),
        "memory_shrinkage": rng.random((B, 1, T, H, W), np.float32),
        "msk_value": rng.standard_normal((B, 1, CV, T, H, W), np.float32),
        "uncert_prob": rng.random((B, 1, H, W), np.float32),
    }
    out = kernel(**dummy)
    print("out", out.shape, out.dtype, float(np.abs(out).mean()))
